# revision 5
# baseline (speedup 1.0000x reference)
"""Mamba SSM block on 8 TRN2 NeuronCores (Bass/Tile, SPMD).

Sharding: d_inner (2048 -> 256/core) across in_proj, conv, dt/B/C projections
and the selective scan (all per-core local). Two small collectives per
512-token chunk:
  - AllReduce of x_dbl projection partials [96, 512] fp32
  - AllGather of the gated scan output yg [256, 512] bf16 (pure copy), after
    which each core computes only its 128 rows of the (host-folded)
    W_c = W_out @ W_out_ssm output projection.

Scan: h[t] = exp(A dt[t]) h[t-1] + (dt[t] x[t]) B[t] via 16 independent
tensor_tensor_scan ops (one per state index) along tokens, chained across
chunks through per-partition `initial` APs. Matmul operands and scan tensors
are bf16 (rel_err ~4e-3 validated vs fp32 reference); conv, activations, dt
and the AllReduce stay fp32.
"""
import numpy as np
import ml_dtypes

import concourse.bass as bass
import concourse.tile as tile
from concourse import bacc, mybir
from concourse.bass_utils import run_bass_kernel_spmd

BFnp = ml_dtypes.bfloat16
F32 = mybir.dt.float32
BF16 = mybir.dt.bfloat16
AF = mybir.ActivationFunctionType
OP = mybir.AluOpType

NC = 8
B, L, DM = 2, 2048, 1024
DI, S, R, KC = 2048, 16, 64, 4
DIL = DI // NC            # 256 d_inner per core
NT = B * L                # 4096 tokens (batch-major)
TC = 512                  # tokens per chunk
NCH = NT // TC            # 8 chunks
EL = DM // NC             # 128 output rows per core
NI = DIL // 128           # 2 partition tiles of local d_inner

_NC_CACHE = {}


def build():
    if "nc" in _NC_CACHE:
        return _NC_CACHE["nc"]
    nc = bacc.Bacc("TRN2", target_bir_lowering=False, debug=False, num_devices=NC)

    # ---- per-core DRAM inputs (host pre-sharded / transposed / casted) ----
    x_t = nc.dram_tensor("x_t", [DM, NT], BF16, kind="ExternalInput")         # replicated
    w_in_x = nc.dram_tensor("w_in_x", [DM, DIL], BF16, kind="ExternalInput")  # W_in[dk,:].T
    w_in_z = nc.dram_tensor("w_in_z", [DM, DIL], BF16, kind="ExternalInput")
    conv_w = nc.dram_tensor("conv_w", [DIL, KC], F32, kind="ExternalInput")
    conv_b = nc.dram_tensor("conv_b", [DIL, 1], F32, kind="ExternalInput")
    w_xp = nc.dram_tensor("w_xp", [DIL, R + 2 * S], F32, kind="ExternalInput")  # W_xp[:,dk].T
    w_dt = nc.dram_tensor("w_dt", [R, DIL], F32, kind="ExternalInput")          # W_dt[dk,:].T
    b_dt = nc.dram_tensor("b_dt", [DIL, 1], F32, kind="ExternalInput")
    a_mat = nc.dram_tensor("a_mat", [DIL, S], F32, kind="ExternalInput")        # -exp(A_log[dk])
    d_vec = nc.dram_tensor("d_vec", [DIL, 1], F32, kind="ExternalInput")
    w_c = nc.dram_tensor("w_c", [DI, EL], BF16, kind="ExternalInput")           # W_c[ek,:].T
    b_o = nc.dram_tensor("b_o", [EL, 1], F32, kind="ExternalInput")
    out = nc.dram_tensor("out", [NCH, EL, TC], F32, kind="ExternalOutput")

    with tile.TileContext(nc) as tc:
        with (
            tc.tile_pool(name="wpool", bufs=1) as wp,     # persistent weights
            tc.tile_pool(name="xpool", bufs=1) as xp,     # streamed x / yg gather
            tc.tile_pool(name="work", bufs=1) as wk,      # DVE-only transients
            tc.tile_pool(name="worka", bufs=2) as wka,    # ACT/DMA-written tiles
            tc.tile_pool(name="keep", bufs=3) as kp,      # xs/g (live across chunk)
            tc.tile_pool(name="scan", bufs=1) as sc,      # big bf16 scan tiles
            tc.tile_pool(name="scana", bufs=2) as sca,    # a (ACT-written, dbl buf)
            tc.tile_pool(name="bcast", bufs=1) as bcp,    # B/C broadcast tiles
            tc.tile_pool(name="state", bufs=1) as st,     # persistent hprev/xtail
            tc.tile_pool(name="psA", bufs=2, space="PSUM") as psA,
            tc.tile_pool(name="psB", bufs=1, space="PSUM") as psB,
            tc.tile_pool(name="dram", bufs=4, space="DRAM") as dr,
        ):
            # ---------- load weights ----------
            winx = wp.tile([128, 8 * NI * 128], BF16, tag="winx")
            winz = wp.tile([128, 8 * NI * 128], BF16, tag="winz")
            for kt in range(8):
                for i in range(NI):
                    nc.sync.dma_start(
                        winx[:, (kt * NI + i) * 128:(kt * NI + i + 1) * 128],
                        w_in_x[kt * 128:(kt + 1) * 128, i * 128:(i + 1) * 128])
                    nc.sync.dma_start(
                        winz[:, (kt * NI + i) * 128:(kt * NI + i + 1) * 128],
                        w_in_z[kt * 128:(kt + 1) * 128, i * 128:(i + 1) * 128])
            wxp = wp.tile([128, NI * (R + 2 * S)], F32, tag="wxp")
            for i in range(NI):
                nc.sync.dma_start(
                    wxp[:, i * (R + 2 * S):(i + 1) * (R + 2 * S)],
                    w_xp[i * 128:(i + 1) * 128, :])
            wdt = wp.tile([R, NI * 128], F32, tag="wdt")
            nc.sync.dma_start(wdt[:], w_dt[:, :])
            wc = wp.tile([128, 16 * EL], BF16, tag="wc")
            for kt in range(16):
                nc.sync.dma_start(
                    wc[:, kt * EL:(kt + 1) * EL],
                    w_c[kt * 128:(kt + 1) * 128, :])
            cw = wp.tile([128, NI * KC], F32, tag="cw")
            cb = wp.tile([128, NI], F32, tag="cb")
            bdt = wp.tile([128, NI], F32, tag="bdt")
            dv = wp.tile([128, NI], F32, tag="dv")
            am = wp.tile([128, NI * S], F32, tag="am")
            for i in range(NI):
                sl = slice(i * 128, (i + 1) * 128)
                nc.sync.dma_start(cw[:, i * KC:(i + 1) * KC], conv_w[sl, :])
                nc.sync.dma_start(cb[:, i:i + 1], conv_b[sl, :])
                nc.sync.dma_start(bdt[:, i:i + 1], b_dt[sl, :])
                nc.sync.dma_start(dv[:, i:i + 1], d_vec[sl, :])
                nc.sync.dma_start(am[:, i * S:(i + 1) * S], a_mat[sl, :])
            bo = wp.tile([EL, 1], F32, tag="bo")
            nc.sync.dma_start(bo[:], b_o[:, :])

            hprev = st.tile([128, NI * S], F32, tag="hprev")
            xtail = st.tile([128, NI * 3], F32, tag="xtail")

            for c in range(NCH):
                t0 = c * TC
                reset = (c % (NCH // B) == 0)

                xck = xp.tile([128, 8 * TC], BF16, tag="xck")
                for kt in range(8):
                    nc.sync.dma_start(
                        xck[:, kt * TC:(kt + 1) * TC],
                        x_t[kt * 128:(kt + 1) * 128, t0:t0 + TC])

                xs_i, g_i = [], []
                for i in range(NI):
                    # ---------- in_proj ----------
                    ps_x = psA.tile([128, TC], F32, tag="psx")
                    ps_z = psA.tile([128, TC], F32, tag="psz")
                    for kt in range(8):
                        wsl = slice((kt * NI + i) * 128, (kt * NI + i + 1) * 128)
                        nc.tensor.matmul(ps_x[:], winx[:, wsl], xck[:, kt * TC:(kt + 1) * TC],
                                         start=(kt == 0), stop=(kt == 7))
                    for kt in range(8):
                        wsl = slice((kt * NI + i) * 128, (kt * NI + i + 1) * 128)
                        nc.tensor.matmul(ps_z[:], winz[:, wsl], xck[:, kt * TC:(kt + 1) * TC],
                                         start=(kt == 0), stop=(kt == 7))

                    # ---------- causal depthwise conv (taps read PSUM) ----------
                    # head buffer: cols 0..2 = prev-chunk tail, cols 3..6 = psum[0:4]
                    head = wk.tile([128, 7], F32, tag="head")
                    if reset:
                        nc.gpsimd.memset(head[:, 0:3], 0.0)
                    else:
                        nc.vector.tensor_copy(head[:, 0:3], xtail[:, i * 3:i * 3 + 3])
                    nc.vector.tensor_copy(head[:, 3:7], ps_x[:, 0:4])
                    nc.vector.tensor_copy(xtail[:, i * 3:i * 3 + 3], ps_x[:, TC - 3:TC])

                    # tokens 3..TC-1: all taps within psum
                    acc0 = wk.tile([128, TC], F32, tag="accA")
                    nc.vector.tensor_scalar_mul(acc0[:, 3:], ps_x[:, 0:TC - 3], cw[:, i * KC:i * KC + 1])
                    acc1 = wk.tile([128, TC], F32, tag="accB")
                    nc.vector.scalar_tensor_tensor(
                        out=acc1[:, 3:], in0=ps_x[:, 1:TC - 2], scalar=cw[:, i * KC + 1:i * KC + 2],
                        in1=acc0[:, 3:], op0=OP.mult, op1=OP.add)
                    acc2 = wk.tile([128, TC], F32, tag="accA")
                    nc.vector.scalar_tensor_tensor(
                        out=acc2[:, 3:], in0=ps_x[:, 2:TC - 1], scalar=cw[:, i * KC + 2:i * KC + 3],
                        in1=acc1[:, 3:], op0=OP.mult, op1=OP.add)
                    u = wk.tile([128, TC], F32, tag="accB")
                    nc.vector.scalar_tensor_tensor(
                        out=u[:, 3:], in0=ps_x[:, 3:TC], scalar=cw[:, i * KC + 3:i * KC + 4],
                        in1=acc2[:, 3:], op0=OP.mult, op1=OP.add)
                    # tokens 0..2 from head buffer
                    nc.vector.tensor_scalar_mul(acc0[:, 0:3], head[:, 0:3], cw[:, i * KC:i * KC + 1])
                    nc.vector.scalar_tensor_tensor(
                        out=acc1[:, 0:3], in0=head[:, 1:4], scalar=cw[:, i * KC + 1:i * KC + 2],
                        in1=acc0[:, 0:3], op0=OP.mult, op1=OP.add)
                    nc.vector.scalar_tensor_tensor(
                        out=acc2[:, 0:3], in0=head[:, 2:5], scalar=cw[:, i * KC + 2:i * KC + 3],
                        in1=acc1[:, 0:3], op0=OP.mult, op1=OP.add)
                    nc.vector.scalar_tensor_tensor(
                        out=u[:, 0:3], in0=head[:, 3:6], scalar=cw[:, i * KC + 3:i * KC + 4],
                        in1=acc2[:, 0:3], op0=OP.mult, op1=OP.add)
                    sgu = wka.tile([128, TC], F32, tag="act1")
                    nc.scalar.activation(sgu[:], u[:], AF.Sigmoid, bias=cb[:, i:i + 1])
                    xs = kp.tile([128, TC], F32, tag="xs")
                    nc.vector.scalar_tensor_tensor(
                        out=xs[:], in0=u[:], scalar=cb[:, i:i + 1], in1=sgu[:],
                        op0=OP.add, op1=OP.mult)
                    xs_i.append(xs)

                    # ---------- z gate ----------
                    zt = wka.tile([128, TC], F32, tag="act2")
                    nc.scalar.copy(zt[:], ps_z[:])
                    sgz = wka.tile([128, TC], F32, tag="act3")
                    nc.scalar.activation(sgz[:], ps_z[:], AF.Sigmoid)
                    g = kp.tile([128, TC], F32, tag="g")
                    nc.vector.tensor_tensor(out=g[:], in0=zt[:], in1=sgz[:], op=OP.mult)
                    g_i.append(g)

                # ---------- x_dbl partial + AllReduce ----------
                ps_xd = psB.tile([R + 2 * S, TC], F32, tag="psxd")
                for i in range(NI):
                    nc.tensor.matmul(ps_xd[:], wxp[:, i * (R + 2 * S):(i + 1) * (R + 2 * S)],
                                     xs_i[i][:], start=(i == 0), stop=(i == NI - 1))
                xd_sb = wka.tile([R + 2 * S, TC], F32, tag="xdsb")
                nc.scalar.copy(xd_sb[:], ps_xd[:])
                xd_part = dr.tile([R + 2 * S, TC], F32, tag="xdp")
                nc.sync.dma_start(xd_part[:], xd_sb[:])
                xd_red = nc.dram_tensor(f"xd_red_{c}", [R + 2 * S, TC], F32, addr_space="Shared")
                nc.gpsimd.collective_compute(
                    "AllReduce", OP.add, replica_groups=[list(range(NC))],
                    ins=[xd_part[:]], outs=[xd_red.ap()])

                # ---------- dtr / B / C ----------
                dtr = wka.tile([R, TC], F32, tag="dtr")
                nc.sync.dma_start(dtr[:], xd_red.ap()[0:R, :])
                b_bc = bcp.tile([128, S * TC], BF16, tag="bbc")
                c_bc = bcp.tile([128, S * TC], BF16, tag="cbc")
                for q in range(4):
                    psl = slice(q * 32, (q + 1) * 32)
                    nc.gpsimd.dma_start(
                        b_bc[psl, :],
                        xd_red.ap()[R:R + S, :].unsqueeze(0).broadcast_to([32, S, TC]))
                    nc.gpsimd.dma_start(
                        c_bc[psl, :],
                        xd_red.ap()[R + S:R + 2 * S, :].unsqueeze(0).broadcast_to([32, S, TC]))

                yg_part = dr.tile([DIL, TC], BF16, tag="ygp")

                for i in range(NI):
                    # ---------- dt = softplus(dt_pre + b_dt) ----------
                    ps_dt = psB.tile([128, TC], F32, tag="psdt")
                    nc.tensor.matmul(ps_dt[:], wdt[:, i * 128:(i + 1) * 128], dtr[:],
                                     start=True, stop=True)
                    edt = wka.tile([128, TC], F32, tag="act1")
                    nc.scalar.activation(edt[:], ps_dt[:], AF.Exp, bias=bdt[:, i:i + 1])
                    dt = wka.tile([128, TC], F32, tag="dtt")
                    nc.scalar.activation(dt[:], edt[:], AF.Ln, bias=1.0)

                    dtx = wk.tile([128, TC], BF16, tag="dtx")
                    nc.vector.tensor_tensor(out=dtx[:], in0=dt[:], in1=xs_i[i][:], op=OP.mult)

                    # ---------- a = exp(A_s * dt) ----------
                    a_t = sca.tile([128, S * TC], BF16, tag="a_t")
                    for s in range(S):
                        nc.scalar.activation(
                            a_t[:, s * TC:(s + 1) * TC], dt[:], AF.Exp,
                            scale=am[:, i * S + s:i * S + s + 1])

                    # ---------- bb = dtx * B ----------
                    bb_t = sc.tile([128, S * TC], BF16, tag="bb_t")
                    nc.vector.tensor_tensor(
                        out=bb_t[:].rearrange("p (s t) -> p s t", s=S),
                        in0=dtx[:].unsqueeze(1).broadcast_to([128, S, TC]),
                        in1=b_bc[:].rearrange("p (s t) -> p s t", s=S), op=OP.mult)

                    # ---------- scan ----------
                    h_t = sc.tile([128, S * TC], BF16, tag="h_t")
                    if reset:
                        nc.gpsimd.memset(hprev[:, i * S:(i + 1) * S], 0.0)
                    for s in range(S):
                        nc.vector.tensor_tensor_scan(
                            h_t[:, s * TC:(s + 1) * TC],
                            a_t[:, s * TC:(s + 1) * TC],
                            bb_t[:, s * TC:(s + 1) * TC],
                            hprev[:, i * S + s:i * S + s + 1],
                            op0=OP.mult, op1=OP.add)
                    for s in range(S):
                        nc.vector.tensor_copy(
                            hprev[:, i * S + s:i * S + s + 1],
                            h_t[:, s * TC + TC - 1:s * TC + TC])

                    # ---------- y = sum_s C*h (tree) ----------
                    hc_t = sc.tile([128, S * TC], BF16, tag="hc_t")
                    nc.vector.tensor_tensor(out=hc_t[:], in0=h_t[:], in1=c_bc[:], op=OP.mult)
                    r1 = sc.tile([128, S * TC // 2], BF16, tag="bb_t")
                    nc.vector.tensor_tensor(out=r1[:], in0=hc_t[:, :S * TC // 2],
                                            in1=hc_t[:, S * TC // 2:], op=OP.add)
                    r2 = sc.tile([128, S * TC // 4], BF16, tag="h_t")
                    nc.vector.tensor_tensor(out=r2[:], in0=r1[:, :S * TC // 4],
                                            in1=r1[:, S * TC // 4:], op=OP.add)
                    r3 = wk.tile([128, S * TC // 8], BF16, tag="r3")
                    nc.vector.tensor_tensor(out=r3[:], in0=r2[:, :S * TC // 8],
                                            in1=r2[:, S * TC // 8:], op=OP.add)
                    y = wk.tile([128, TC], F32, tag="y")
                    nc.vector.tensor_tensor(out=y[:], in0=r3[:, :TC], in1=r3[:, TC:], op=OP.add)

                    # ---------- gate ----------
                    yD = wk.tile([128, TC], F32, tag="yD")
                    nc.vector.scalar_tensor_tensor(
                        out=yD[:], in0=xs_i[i][:], scalar=dv[:, i:i + 1], in1=y[:],
                        op0=OP.mult, op1=OP.add)
                    yg = wk.tile([128, TC], BF16, tag="yg")
                    nc.vector.tensor_tensor(out=yg[:], in0=yD[:], in1=g_i[i][:], op=OP.mult)
                    nc.sync.dma_start(yg_part[i * 128:(i + 1) * 128, :], yg[:])

                # ---------- AllGather yg; out proj ----------
                yg_full = nc.dram_tensor(f"yg_full_{c}", [DI, TC], BF16, addr_space="Shared")
                nc.gpsimd.collective_compute(
                    "AllGather", OP.bypass, replica_groups=[list(range(NC))],
                    ins=[yg_part[:]], outs=[yg_full.ap()])
                ps_o = psB.tile([EL, TC], F32, tag="pso")
                for half in range(2):
                    ygs = xp.tile([128, 8 * TC], BF16, tag="ygs")
                    for j in range(8):
                        kt = half * 8 + j
                        nc.sync.dma_start(ygs[:, j * TC:(j + 1) * TC],
                                          yg_full.ap()[kt * 128:(kt + 1) * 128, :])
                    for j in range(8):
                        kt = half * 8 + j
                        nc.tensor.matmul(ps_o[:], wc[:, kt * EL:(kt + 1) * EL],
                                         ygs[:, j * TC:(j + 1) * TC],
                                         start=(kt == 0), stop=(kt == 15))
                o_sb = wka.tile([EL, TC], F32, tag="osb")
                nc.scalar.activation(o_sb[:], ps_o[:], AF.Identity, bias=bo[:])
                nc.sync.dma_start(out[c, :, :], o_sb[:])

    nc.compile()
    _NC_CACHE["nc"] = nc
    return nc


def _prep_inputs(inputs):
    x = np.ascontiguousarray(np.asarray(inputs["x"], np.float32))
    W_in = np.asarray(inputs["W_in"], np.float32)
    conv_w = np.asarray(inputs["conv_w"], np.float32)
    conv_b = np.asarray(inputs["conv_b"], np.float32)
    W_xp = np.asarray(inputs["W_xp"], np.float32)
    W_dt = np.asarray(inputs["W_dt"], np.float32)
    b_dt = np.asarray(inputs["b_dt"], np.float32)
    A_log = np.asarray(inputs["A_log"], np.float32)
    D = np.asarray(inputs["D"], np.float32)
    W_out_ssm = np.asarray(inputs["W_out_ssm"], np.float32)
    W_out = np.asarray(inputs["W_out"], np.float32)
    b_out = np.asarray(inputs["b_out"], np.float32)

    A = -np.exp(A_log)
    W_c = (W_out.astype(np.float64) @ W_out_ssm.astype(np.float64)).astype(np.float32)
    x_t = np.ascontiguousarray(x.reshape(NT, DM).T.astype(BFnp))  # [DM, NT] bf16

    in_maps = []
    for k in range(NC):
        dsl = slice(k * DIL, (k + 1) * DIL)
        esl = slice(k * EL, (k + 1) * EL)
        in_maps.append({
            "x_t": x_t,
            "w_in_x": np.ascontiguousarray(W_in[dsl, :].T.astype(BFnp)),
            "w_in_z": np.ascontiguousarray(
                W_in[DI + k * DIL: DI + (k + 1) * DIL, :].T.astype(BFnp)),
            "conv_w": np.ascontiguousarray(conv_w[dsl, 0, :]),
            "conv_b": np.ascontiguousarray(conv_b[dsl][:, None]),
            "w_xp": np.ascontiguousarray(W_xp[:, dsl].T),
            "w_dt": np.ascontiguousarray(W_dt[dsl, :].T),
            "b_dt": np.ascontiguousarray(b_dt[dsl][:, None]),
            "a_mat": np.ascontiguousarray(A[dsl, :]),
            "d_vec": np.ascontiguousarray(D[dsl][:, None]),
            "w_c": np.ascontiguousarray(W_c[esl, :].T.astype(BFnp)),
            "b_o": np.ascontiguousarray(b_out[esl][:, None]),
        })
    return in_maps


def _assemble(results):
    full = np.zeros((DM, NT), np.float32)
    for k in range(NC):
        o = results[k]["out"]  # [NCH, EL, TC]
        for c in range(NCH):
            full[k * EL:(k + 1) * EL, c * TC:(c + 1) * TC] = o[c]
    return np.ascontiguousarray(full.T).reshape(B, L, DM)


def kernel(**inputs):
    nc = build()
    in_maps = _prep_inputs(inputs)
    res = run_bass_kernel_spmd(nc, in_maps, core_ids=list(range(NC)))
    return _assemble(res.results)


def kernel_sim(**inputs):
    """Run through MultiCoreSim instead of HW (for debugging)."""
    from concourse.bass_interp import MultiCoreSim
    nc = build()
    in_maps = _prep_inputs(inputs)
    sim = MultiCoreSim(nc, num_cores=NC)
    for k in range(NC):
        for name, arr in in_maps[k].items():
            sim.cores[k].tensor(name)[:] = arr
    sim.simulate(check_with_hw=False)
    results = [{"out": sim.cores[k].tensor("out").copy()} for k in range(NC)]
    return _assemble(results)
